# revision 4
# baseline (speedup 1.0000x reference)
"""DOA pattern loss kernel for Trainium2 (8 NeuronCores, SPMD) — v2.

Computes min_r sum_a (possible_phases[r, a] - phases[a])^2 over a
[1_000_000, 32] codebook, returning the scalar min.

v2 strategy (memory-bound problem):
  - Codebook quantized HOST-side to uint8 (u = round(x / c), c = 2pi/256):
    halves HBM traffic vs fp16 (4 MB/core).  Exact distances to the
    quantized codebook are computed on-device in integer units
    (D_u = sum (u - q)^2, q = phases/c); host rescales by c^2.  Measured
    end-to-end rel err ~3e-3 vs the 2e-2 gate (argmin unchanged).
  - Rows sharded over 8 cores; each core's shard is split into 4
    row-quarters stacked on the partition axis: partition 32*quarter +
    antenna, free dim = row position (contiguous DMA, antennas land on
    partitions for the PE reduction).
  - Elementwise distance terms are computed in ONE pass per engine and
    split across engines at PSUM-bank granularity (4 chunks of 512
    positions per bank, PATTERN maps bank -> engine):
      'A' banks on ScalarE:  (u - q)^2        = Square(1.0*u + (-q))
      'D' banks on VectorE:  (u - 2q)*u       = scalar_tensor_tensor
    The D/P form omits the per-row-constant sum_a q_a^2 (= ||q||^2), so
    their row sums are uniformly offset by -||q||^2; mins are tracked in
    two groups and reconciled host-side.  |u(u-2q)| <= 65025 < fp16 max.
  - TensorE reduces antenna groups with a stationary ones matrix
    B[128, 32] (B[p, m] = 1 iff p//32 == m//8): 4 matmuls per PSUM bank at
    partition offsets 0/32/64/96 pack 8192 row sums per bank.
  - PSUM is allocated as 4-bank tiles [128, 2048]; VectorE min-reduces
    maximal same-group bank runs in ONE tensor_reduce (up to [128, 2048])
    to amortize per-instruction overhead.
  - Final min per group -> [128, 2] -> DRAM.  Host: min over cores of
    min(group_A, group_D + ||q||^2) * c^2.
"""

import numpy as np

P = 128          # SBUF partitions
A = 32           # antennas
NQ = 4           # row-quarters stacked on the partition axis
CHUNK = 512      # matmul rhs free size = one PSUM bank of fp32
BANKCH = 4       # chunks per PSUM bank
TBANKS = 4       # banks per PSUM tile
NCORES = 8

QPOS = 31744     # row positions per quarter per core = 62 * 512
RC = NQ * QPOS   # rows per core = 126976 (8 cores = 1015808 >= 1e6, padded)
W = 8192         # positions per DMA tile (1 MB uint8)

# engine per full PSUM bank (A=ScalarE square, D=VectorE stt, P=GpSimd
# ts+tt -> group A); the 2-chunk ragged bank is always ScalarE.  Keep
# same-letter banks contiguous within 4-bank PSUM tiles so reduce runs
# stay long, but interleave at tile granularity so engines overlap.
# NOTE: GpSimd ('P') elementwise ops measured ~12us/instruction on real HW
# (Q7 software) despite favorable cost-model rates — never schedule them.
PATTERN = "AAAADDDDAAAADAA"

_cache: dict = {}


def build_nc(qpos: int = QPOS, w: int = W, reps: int = 1,
             pattern: str = PATTERN,
             xbufs: int = 8, dbufs: int = 6, pbufs: int = 4,
             tbanks: int = 2, dma_alt: bool = False):
    """Build the single-core Bass program (same NEFF runs SPMD on all cores)."""
    from contextlib import ExitStack

    import concourse.bacc as bacc
    import concourse.tile as tile
    from concourse import mybir

    f16 = mybir.dt.float16
    f32 = mybir.dt.float32
    u8 = mybir.dt.uint8
    nc = bacc.Bacc("TRN2", target_bir_lowering=False)

    assert qpos % CHUNK == 0 and w % (CHUNK * BANKCH) == 0
    nch = qpos // CHUNK                      # chunks per rep (62)
    nfull = (nch // BANKCH) * BANKCH         # chunks in full banks (60)
    nbank = (nch + BANKCH - 1) // BANKCH     # banks per rep (16)
    assert len(pattern) >= nfull // BANKCH, (pattern, nfull)

    def ctype(c):  # engine of chunk c
        if c >= nfull:
            return "A"  # ragged bank -> ScalarE
        return pattern[c // BANKCH]

    def bgroup(b):  # min-group of bank b: Pool's (u-q)^2 is group A
        return "D" if ctype(b * BANKCH) == "D" else "A"

    # reduce runs: maximal same-group runs of full banks within each PSUM
    # tile; the ragged bank reduces alone over its written partitions.
    TB = tbanks
    pbufs = min(pbufs, 8 // TB)
    npt = (nbank + TB - 1) // TB
    runs = []  # (pt, b_lo, b_hi, group, npart_of_last_bank)
    for pt in range(npt):
        b0, b1 = pt * TB, min((pt + 1) * TB, nbank)
        b = b0
        while b < b1:
            isragged = (b + 1) * BANKCH > nch
            if isragged:
                runs.append((pt, b, b + 1, bgroup(b), 32 * (nch - b * BANKCH)))
                b += 1
                continue
            e = b + 1
            while e < b1 and (e + 1) * BANKCH <= nch and bgroup(e) == bgroup(b):
                e += 1
            runs.append((pt, b, e, bgroup(b), P))
            b = e
    ca_rep = sum(1 for r in runs if r[3] == "A")
    cd_rep = sum(1 for r in runs if r[3] == "D")
    n_a = max(ca_rep * reps, 1)
    n_d = max(cd_rep * reps, 1)

    BIG = 3.0e38  # +inf stand-in (finite, far above any real distance)

    with tile.TileContext(nc) as tc:
        with ExitStack() as ctx:
            singles = ctx.enter_context(tc.tile_pool(name="singles", bufs=1))
            xpool = ctx.enter_context(tc.tile_pool(name="xin", bufs=xbufs))
            dpool = ctx.enter_context(tc.tile_pool(name="d2", bufs=dbufs))
            if any(t == "P" in (True,) for t in ()) or "P" in pattern[: nfull // BANKCH]:
                ypool = ctx.enter_context(tc.tile_pool(name="y", bufs=3))
            ppool = ctx.enter_context(tc.tile_pool(name="ps", bufs=pbufs, space="PSUM"))

            cb = nc.dram_tensor("cb", [P, qpos], u8, kind="ExternalInput")
            negq = nc.dram_tensor("negq", [P, 1], f32, kind="ExternalInput")
            neg2q = nc.dram_tensor("neg2q", [P, 1], f32, kind="ExternalInput")
            bmat = nc.dram_tensor("bmat", [P, A], f16, kind="ExternalInput")
            out = nc.dram_tensor("out", [P, 2], f32, kind="ExternalOutput")

            negq_s = singles.tile([P, 1], f32)
            nc.sync.dma_start(out=negq_s[:, :], in_=negq[:, :])
            neg2q_s = singles.tile([P, 1], f32)
            nc.sync.dma_start(out=neg2q_s[:, :], in_=neg2q[:, :])
            b_s = singles.tile([P, A], f16)
            nc.sync.dma_start(out=b_s[:, :], in_=bmat[:, :])
            stage = singles.tile([P, n_a + n_d], f32)
            nc.vector.memset(stage[:, :], BIG)
            final = singles.tile([P, 2], f32)

            ga = 0          # next stage column for group A (cols [0, n_a))
            gd = n_a        # next stage column for group D

            wch = w // CHUNK  # chunks per DMA tile

            for rep in range(reps):
                x_tiles = []
                d_tiles = []
                nt = (nch + wch - 1) // wch
                for ti in range(nt):
                    wt = min(w, qpos - ti * w)
                    x = xpool.tile([P, w], u8, tag="x")
                    eng = nc.scalar if (dma_alt and ti % 2) else nc.sync
                    eng.dma_start(out=x[:, :wt], in_=cb[:, ti * w : ti * w + wt])
                    x_tiles.append(x)
                    d2 = dpool.tile([P, w], f16, tag="d2")
                    d_tiles.append(d2)

                # elementwise pass, one instruction per bank
                for b in range(nbank):
                    c0 = b * BANKCH
                    c1 = min(c0 + BANKCH, nch)
                    ti = (c0 * CHUNK) // w
                    lo = c0 * CHUNK - ti * w
                    hi = c1 * CHUNK - ti * w
                    t = ctype(c0)
                    x, d2 = x_tiles[ti], d_tiles[ti]
                    if t == "A":
                        nc.scalar.activation(
                            d2[:, lo:hi],
                            x[:, lo:hi],
                            mybir.ActivationFunctionType.Square,
                            bias=negq_s[:, :],
                            scale=1.0,
                        )
                    elif t == "D":
                        nc.vector.scalar_tensor_tensor(
                            d2[:, lo:hi], x[:, lo:hi], neg2q_s[:, :], x[:, lo:hi],
                            mybir.AluOpType.add, mybir.AluOpType.mult,
                        )
                    else:
                        # Pool: TensorScalarPtr with 2 ops is illegal on Pool;
                        # two legal ops instead: y = u - q, d2 = y*y (group A)
                        y = ypool.tile([P, BANKCH * CHUNK], f16, tag="y")
                        yw = hi - lo
                        nc.gpsimd.tensor_scalar(
                            y[:, :yw], x[:, lo:hi], negq_s[:, :], None,
                            mybir.AluOpType.add,
                        )
                        nc.gpsimd.tensor_tensor(
                            d2[:, lo:hi], y[:, :yw], y[:, :yw], mybir.AluOpType.mult,
                        )

                # matmuls into 4-bank PSUM tiles; reduces fire per run
                pt_tiles = {}
                ri = 0
                for b in range(nbank):
                    pt = b // TB
                    if pt not in pt_tiles:
                        ps4 = ppool.tile([P, TB * CHUNK], f32, tag="ps")
                        pt_tiles[pt] = ps4
                    ps4 = pt_tiles[pt]
                    bb = b % TB
                    c0 = b * BANKCH
                    c1 = min(c0 + BANKCH, nch)
                    ti = (c0 * CHUNK) // w
                    d2 = d_tiles[ti]
                    for jj in range(c1 - c0):
                        lo = (c0 + jj) * CHUNK - ti * w
                        nc.tensor.matmul(
                            ps4[32 * jj : 32 * (jj + 1), bb * CHUNK : (bb + 1) * CHUNK],
                            b_s[:, :],
                            d2[:, lo : lo + CHUNK],
                            start=True,
                            stop=True,
                            tile_position=(0, 32 * jj),
                        )
                    # emit reduce runs whose last bank just completed
                    while ri < len(runs) and runs[ri][2] - 1 == b:
                        pt_r, b_lo, b_hi, g, npart = runs[ri]
                        ri += 1
                        ps_r = pt_tiles[pt_r]
                        f_lo = (b_lo - pt_r * TB) * CHUNK
                        f_hi = (b_hi - pt_r * TB) * CHUNK
                        col = ga if g == "A" else gd
                        nc.vector.tensor_reduce(
                            out=stage[:npart, col : col + 1],
                            in_=ps_r[:npart, f_lo:f_hi],
                            axis=mybir.AxisListType.X,
                            op=mybir.AluOpType.min,
                        )
                        if g == "A":
                            ga += 1
                        else:
                            gd += 1
                assert ri == len(runs)

            assert ga <= n_a and gd <= n_a + n_d, (ga, gd, n_a, n_d)
            nc.vector.tensor_reduce(
                out=final[:, 0:1],
                in_=stage[:, :n_a],
                axis=mybir.AxisListType.X,
                op=mybir.AluOpType.min,
            )
            nc.vector.tensor_reduce(
                out=final[:, 1:2],
                in_=stage[:, n_a:],
                axis=mybir.AxisListType.X,
                op=mybir.AluOpType.min,
            )
            nc.sync.dma_start(out=out[:, :], in_=final[:, :])

    nc.compile()
    return nc


C_SCALE = 2.0 * np.pi / 256.0


def make_in_maps(possible_phases: np.ndarray, phases: np.ndarray, qpos: int = QPOS):
    """Quantize to uint8, shard + quarter-transpose; build per-core inputs."""
    rc = NQ * qpos
    rpad = NCORES * rc
    pp = np.asarray(possible_phases, dtype=np.float32)
    u = np.clip(np.rint(pp * (1.0 / C_SCALE)), 0, 255).astype(np.uint8)
    r = u.shape[0]
    assert rpad >= r, (rpad, r)
    if rpad > r:
        u = np.concatenate([u, u[: rpad - r]], axis=0)  # duplicate rows: min unchanged

    q = (np.asarray(phases, dtype=np.float32).reshape(A) / C_SCALE).astype(np.float32)
    negq = np.tile(-q, NQ).reshape(P, 1).astype(np.float32)
    neg2q = (2.0 * negq).astype(np.float32)
    bmat = np.kron(
        np.eye(NQ, dtype=np.float16), np.ones((A, A // NQ), dtype=np.float16)
    )  # [128, 32], B[p, m] = 1 iff p//32 == m//8

    in_maps = []
    for c in range(NCORES):
        shard = u[c * rc : (c + 1) * rc]  # [rc, 32]
        cbq = np.ascontiguousarray(
            shard.reshape(NQ, qpos, A).transpose(0, 2, 1).reshape(P, qpos)
        )
        in_maps.append({"cb": cbq, "negq": negq, "neg2q": neg2q, "bmat": bmat})
    return in_maps


def kernel(possible_phases: np.ndarray, phases: np.ndarray) -> np.ndarray:
    from concourse.bass_utils import run_bass_kernel_spmd

    if "nc" not in _cache:
        _cache["nc"] = build_nc()
    in_maps = make_in_maps(possible_phases, phases)
    res = run_bass_kernel_spmd(_cache["nc"], in_maps, core_ids=list(range(NCORES)))
    outs = np.stack([res.results[c]["out"] for c in range(NCORES)])  # [8, 128, 2]
    q = np.asarray(phases, dtype=np.float32).reshape(A) / C_SCALE
    qq = float((q.astype(np.float64) ** 2).sum())
    min_a = float(outs[:, :, 0].min())
    min_d = float(outs[:, :, 1].min()) + qq
    return np.float32(C_SCALE * C_SCALE * min(min_a, min_d))


# revision 6
# speedup vs baseline: 1.3711x; 1.3711x over previous
"""DOA pattern loss kernel for Trainium2 (8 NeuronCores, SPMD) — v2.

Computes min_r sum_a (possible_phases[r, a] - phases[a])^2 over a
[1_000_000, 32] codebook, returning the scalar min.

v2 strategy (memory-bound problem):
  - Codebook quantized HOST-side to uint8 (u = round(x / c), c = 2pi/256):
    halves HBM traffic vs fp16 (4 MB/core).  Exact distances to the
    quantized codebook are computed on-device in integer units
    (D_u = sum (u - q)^2, q = phases/c); host rescales by c^2.  Measured
    end-to-end rel err ~3e-3 vs the 2e-2 gate (argmin unchanged).
  - Rows sharded over 8 cores; each core's shard is split into 4
    row-quarters stacked on the partition axis: partition 32*quarter +
    antenna, free dim = row position (contiguous DMA, antennas land on
    partitions for the PE reduction).
  - Elementwise distance terms are computed in ONE pass per engine and
    split across engines at PSUM-bank granularity (4 chunks of 512
    positions per bank, PATTERN maps bank -> engine):
      'A' banks on ScalarE:  (u - q)^2        = Square(1.0*u + (-q))
      'D' banks on VectorE:  (u - 2q)*u       = scalar_tensor_tensor
    The D form omits the per-row-constant sum_a q_a^2 (= ||q||^2), so
    their row sums are uniformly offset by -||q||^2; mins are tracked in
    two groups and reconciled host-side.  |u(u-2q)| <= 65025 < fp16 max.
  - TensorE reduces antenna groups with a stationary ones matrix
    B[128, 32] (B[p, m] = 1 iff p//32 == m//8): 4 matmuls per PSUM bank at
    partition offsets 0/32/64/96 pack 8192 row sums per bank.
  - PSUM is allocated as multi-bank tiles (tbanks*512 fp32 wide); VectorE
    min-reduces maximal same-group bank runs in one tensor_reduce to
    amortize per-instruction overhead.
  - Final min per group -> [128, 2] -> DRAM.  Host: min over cores of
    min(group_A, group_D + ||q||^2) * c^2.
"""

import numpy as np

P = 128          # SBUF partitions
A = 32           # antennas
NQ = 4           # row-quarters stacked on the partition axis
CHUNK = 512      # matmul rhs free size = one PSUM bank of fp32
BANKCH = 4       # chunks per PSUM bank
TBANKS = 4       # banks per PSUM tile
NCORES = 8

QPOS = 31744     # row positions per quarter per core = 62 * 512
RC = NQ * QPOS   # rows per core = 126976 (8 cores = 1015808 >= 1e6, padded)
W = 8192         # positions per DMA tile (1 MB uint8)

# engine per full PSUM bank (A=ScalarE square, D=VectorE stt, P=GpSimd
# ts+tt -> group A); the 2-chunk ragged bank is always ScalarE.  Keep
# same-letter banks contiguous within 4-bank PSUM tiles so reduce runs
# stay long, but interleave at tile granularity so engines overlap.
# NOTE: GpSimd ('P') elementwise ops measured ~12us/instruction on real HW
# (Q7 software) despite favorable cost-model rates — never schedule them.
PATTERN = "AAAADDDDAAAADAA"

_cache: dict = {}


def build_nc(qpos: int = QPOS, w: int = W, reps: int = 1,
             pattern: str = PATTERN,
             xbufs: int = 8, dbufs: int = 6, pbufs: int = 4,
             tbanks: int = 2, dma_alt: bool = False):
    """Build the single-core Bass program (same NEFF runs SPMD on all cores)."""
    from contextlib import ExitStack

    import concourse.bacc as bacc
    import concourse.tile as tile
    from concourse import mybir

    f16 = mybir.dt.float16
    f32 = mybir.dt.float32
    u8 = mybir.dt.uint8
    nc = bacc.Bacc("TRN2", target_bir_lowering=False)

    assert qpos % CHUNK == 0 and w % (CHUNK * BANKCH) == 0
    nch = qpos // CHUNK                      # chunks per rep (62)
    nfull = (nch // BANKCH) * BANKCH         # chunks in full banks (60)
    nbank = (nch + BANKCH - 1) // BANKCH     # banks per rep (16)
    assert len(pattern) >= nfull // BANKCH, (pattern, nfull)

    def ctype(c):  # engine of chunk c
        if c >= nfull:
            return "A"  # ragged bank -> ScalarE
        return pattern[c // BANKCH]

    def bgroup(b):  # min-group of bank b: Pool's (u-q)^2 is group A
        return "D" if ctype(b * BANKCH) == "D" else "A"

    # reduce runs: maximal same-group runs of full banks within each PSUM
    # tile; the ragged bank reduces alone over its written partitions.
    TB = tbanks
    pbufs = min(pbufs, 8 // TB)
    npt = (nbank + TB - 1) // TB
    runs = []  # (pt, b_lo, b_hi, group, npart_of_last_bank)
    for pt in range(npt):
        b0, b1 = pt * TB, min((pt + 1) * TB, nbank)
        b = b0
        while b < b1:
            isragged = (b + 1) * BANKCH > nch
            if isragged:
                runs.append((pt, b, b + 1, bgroup(b), 32 * (nch - b * BANKCH)))
                b += 1
                continue
            e = b + 1
            while e < b1 and (e + 1) * BANKCH <= nch and bgroup(e) == bgroup(b):
                e += 1
            runs.append((pt, b, e, bgroup(b), P))
            b = e
    ca_rep = sum(1 for r in runs if r[3] == "A")
    cd_rep = sum(1 for r in runs if r[3] == "D")
    n_a = max(ca_rep * reps, 1)
    n_d = max(cd_rep * reps, 1)

    BIG = 3.0e38  # +inf stand-in (finite, far above any real distance)

    with tile.TileContext(nc) as tc:
        with ExitStack() as ctx:
            singles = ctx.enter_context(tc.tile_pool(name="singles", bufs=1))
            xpool = ctx.enter_context(tc.tile_pool(name="xin", bufs=xbufs))
            dpool = ctx.enter_context(tc.tile_pool(name="d2", bufs=dbufs))
            if "P" in pattern[: nfull // BANKCH]:
                ypool = ctx.enter_context(tc.tile_pool(name="y", bufs=3))
            ppool = ctx.enter_context(tc.tile_pool(name="ps", bufs=pbufs, space="PSUM"))

            cb = nc.dram_tensor("cb", [P, qpos], u8, kind="ExternalInput")
            negq = nc.dram_tensor("negq", [P, 1], f32, kind="ExternalInput")
            neg2q = nc.dram_tensor("neg2q", [P, 1], f32, kind="ExternalInput")
            bmat = nc.dram_tensor("bmat", [P, A], f16, kind="ExternalInput")
            out = nc.dram_tensor("out", [P, 2], f32, kind="ExternalOutput")

            negq_s = singles.tile([P, 1], f32)
            nc.sync.dma_start(out=negq_s[:, :], in_=negq[:, :])
            neg2q_s = singles.tile([P, 1], f32)
            nc.sync.dma_start(out=neg2q_s[:, :], in_=neg2q[:, :])
            b_s = singles.tile([P, A], f16)
            nc.sync.dma_start(out=b_s[:, :], in_=bmat[:, :])
            stage = singles.tile([P, n_a + n_d], f32)
            nc.vector.memset(stage[:, :], BIG)
            final = singles.tile([P, 2], f32)

            ga = 0          # next stage column for group A (cols [0, n_a))
            gd = n_a        # next stage column for group D

            wch = w // CHUNK  # chunks per DMA tile

            for rep in range(reps):
                x_tiles = []
                d_tiles = []
                nt = (nch + wch - 1) // wch
                for ti in range(nt):
                    wt = min(w, qpos - ti * w)
                    x = xpool.tile([P, w], u8, tag="x")
                    eng = nc.scalar if (dma_alt and ti % 2) else nc.sync
                    eng.dma_start(out=x[:, :wt], in_=cb[:, ti * w : ti * w + wt])
                    x_tiles.append(x)
                    d2 = dpool.tile([P, w], f16, tag="d2")
                    d_tiles.append(d2)

                # elementwise pass, one instruction per bank
                for b in range(nbank):
                    c0 = b * BANKCH
                    c1 = min(c0 + BANKCH, nch)
                    ti = (c0 * CHUNK) // w
                    lo = c0 * CHUNK - ti * w
                    hi = c1 * CHUNK - ti * w
                    t = ctype(c0)
                    x, d2 = x_tiles[ti], d_tiles[ti]
                    if t == "A":
                        nc.scalar.activation(
                            d2[:, lo:hi],
                            x[:, lo:hi],
                            mybir.ActivationFunctionType.Square,
                            bias=negq_s[:, :],
                            scale=1.0,
                        )
                    elif t == "D":
                        nc.vector.scalar_tensor_tensor(
                            d2[:, lo:hi], x[:, lo:hi], neg2q_s[:, :], x[:, lo:hi],
                            mybir.AluOpType.add, mybir.AluOpType.mult,
                        )
                    else:
                        # Pool: TensorScalarPtr with 2 ops is illegal on Pool;
                        # two legal ops instead: y = u - q, d2 = y*y (group A)
                        y = ypool.tile([P, BANKCH * CHUNK], f16, tag="y")
                        yw = hi - lo
                        nc.gpsimd.tensor_scalar(
                            y[:, :yw], x[:, lo:hi], negq_s[:, :], None,
                            mybir.AluOpType.add,
                        )
                        nc.gpsimd.tensor_tensor(
                            d2[:, lo:hi], y[:, :yw], y[:, :yw], mybir.AluOpType.mult,
                        )

                # matmuls into 4-bank PSUM tiles; reduces fire per run
                pt_tiles = {}
                ri = 0
                for b in range(nbank):
                    pt = b // TB
                    if pt not in pt_tiles:
                        ps4 = ppool.tile([P, TB * CHUNK], f32, tag="ps")
                        pt_tiles[pt] = ps4
                    ps4 = pt_tiles[pt]
                    bb = b % TB
                    c0 = b * BANKCH
                    c1 = min(c0 + BANKCH, nch)
                    ti = (c0 * CHUNK) // w
                    d2 = d_tiles[ti]
                    for jj in range(c1 - c0):
                        lo = (c0 + jj) * CHUNK - ti * w
                        nc.tensor.matmul(
                            ps4[32 * jj : 32 * (jj + 1), bb * CHUNK : (bb + 1) * CHUNK],
                            b_s[:, :],
                            d2[:, lo : lo + CHUNK],
                            start=True,
                            stop=True,
                            tile_position=(0, 32 * jj),
                        )
                    # emit reduce runs whose last bank just completed
                    while ri < len(runs) and runs[ri][2] - 1 == b:
                        pt_r, b_lo, b_hi, g, npart = runs[ri]
                        ri += 1
                        ps_r = pt_tiles[pt_r]
                        f_lo = (b_lo - pt_r * TB) * CHUNK
                        f_hi = (b_hi - pt_r * TB) * CHUNK
                        col = ga if g == "A" else gd
                        nc.vector.tensor_reduce(
                            out=stage[:npart, col : col + 1],
                            in_=ps_r[:npart, f_lo:f_hi],
                            axis=mybir.AxisListType.X,
                            op=mybir.AluOpType.min,
                        )
                        if g == "A":
                            ga += 1
                        else:
                            gd += 1
                assert ri == len(runs)

            assert ga <= n_a and gd <= n_a + n_d, (ga, gd, n_a, n_d)
            nc.vector.tensor_reduce(
                out=final[:, 0:1],
                in_=stage[:, :n_a],
                axis=mybir.AxisListType.X,
                op=mybir.AluOpType.min,
            )
            nc.vector.tensor_reduce(
                out=final[:, 1:2],
                in_=stage[:, n_a:],
                axis=mybir.AxisListType.X,
                op=mybir.AluOpType.min,
            )
            nc.sync.dma_start(out=out[:, :], in_=final[:, :])

    nc.compile()
    return nc


C_SCALE = 2.0 * np.pi / 256.0


def make_in_maps(possible_phases: np.ndarray, phases: np.ndarray, qpos: int = QPOS):
    """Quantize to uint8, shard + quarter-transpose; build per-core inputs."""
    rc = NQ * qpos
    rpad = NCORES * rc
    pp = np.asarray(possible_phases, dtype=np.float32)
    u = np.clip(np.rint(pp * (1.0 / C_SCALE)), 0, 255).astype(np.uint8)
    r = u.shape[0]
    assert rpad >= r, (rpad, r)
    if rpad > r:
        u = np.concatenate([u, u[: rpad - r]], axis=0)  # duplicate rows: min unchanged

    q = (np.asarray(phases, dtype=np.float32).reshape(A) / C_SCALE).astype(np.float32)
    negq = np.tile(-q, NQ).reshape(P, 1).astype(np.float32)
    neg2q = (2.0 * negq).astype(np.float32)
    bmat = np.kron(
        np.eye(NQ, dtype=np.float16), np.ones((A, A // NQ), dtype=np.float16)
    )  # [128, 32], B[p, m] = 1 iff p//32 == m//8

    in_maps = []
    for c in range(NCORES):
        shard = u[c * rc : (c + 1) * rc]  # [rc, 32]
        cbq = np.ascontiguousarray(
            shard.reshape(NQ, qpos, A).transpose(0, 2, 1).reshape(P, qpos)
        )
        in_maps.append({"cb": cbq, "negq": negq, "neg2q": neg2q, "bmat": bmat})
    return in_maps


def kernel(possible_phases: np.ndarray, phases: np.ndarray) -> np.ndarray:
    from concourse.bass_utils import run_bass_kernel_spmd

    if "nc" not in _cache:
        _cache["nc"] = build_nc()
    in_maps = make_in_maps(possible_phases, phases)
    res = run_bass_kernel_spmd(_cache["nc"], in_maps, core_ids=list(range(NCORES)))
    outs = np.stack([res.results[c]["out"] for c in range(NCORES)])  # [8, 128, 2]
    q = np.asarray(phases, dtype=np.float32).reshape(A) / C_SCALE
    qq = float((q.astype(np.float64) ** 2).sum())
    min_a = float(outs[:, :, 0].min())
    min_d = float(outs[:, :, 1].min()) + qq
    return np.float32(C_SCALE * C_SCALE * min(min_a, min_d))
